# revision 1
# baseline (speedup 1.0000x reference)
"""LocalPatchAttention Trainium2 kernel.

Data-parallel over batch B=8 across 8 NeuronCores (one image per core).
q and out live in DRAM as [128, 32768] with partitions = (channel,
row-parity): partition p<64 = channel p of even rows, p>=64 = channel p-64
of odd rows; host packs/unpacks with cheap numpy reshapes.

Per 2-row pair (128 pairs per core):
  - [128,512] q load per 2 pairs, plus a [64,512] load of the odd-row half
    so every PE contraction runs from partition base 0.
  - GPSIMD makes bf16 q and q^2 copies; PE transposes 128-px chunks into a
    shared PSUM tile whose tail columns hold per-pixel sum(q)/64 and
    sum(q^2)/64 from N=1 matmuls (no bn_stats).
  - rsqrt(var+eps) via bit-trick + 1 Newton step: shifts/int-ALU on DVE,
    multiplies on GPSIMD ([128,4] per pair, all four chunks at once).
  - normalize on DVE (2-op tensor_scalar, per-partition mean/rsqrt APs) ->
    xh bf16; PE transpose-back into xhT_ps [64,512]; Act copy to SBUF; one
    logits matmul with host-folded A = scale*(g*qW^T)K^T; Act Sigmoid
    (folded bias) split in halves; srow = sig * V on GPSIMD via stride-0
    broadcast AP.
  - 3x3 conv in fp8e4m3 with DoubleRow perf mode: srows live in a 13-slot
    fp8 ring (slot r%12, slot 12 duplicating r%12==0 rows) so each matmul
    contracts TWO vertical taps at once (12 DoubleRow matmuls per 2 pairs,
    0.5 cy/row); weights are host-scaled x32 and V by 1/32 to sit in fp8's
    normal range; K=1 bias matmul opens the PSUM group; residual q add +
    PSUM drain fused in one DVE op per pair; one [128,512] store per
    2 pairs.
Act engine only runs Copy/Identity/Sigmoid/Square -> one act table load.
V path (once per core): LN stats via 1/128-matmuls, DVE/GPSIMD rsqrt rows,
K=1 broadcast matmuls, vwf matmul, Act bias add.
"""

import numpy as np
import ml_dtypes

import concourse.bass as bass
import concourse.bacc as bacc
import concourse.tile as tile
from concourse import mybir
from concourse.bass_utils import run_bass_kernel_spmd

F32 = mybir.dt.float32
F32R = mybir.dt.float32r
BF16 = mybir.dt.bfloat16
U32 = mybir.dt.uint32
I32 = mybir.dt.int32
AF = mybir.ActivationFunctionType
ALU = mybir.AluOpType
EPS = 1e-5
MAGIC = 0x5F3759DF
NPBF16 = ml_dtypes.bfloat16

_CACHE = {}


def _build_nc():
    nc = bacc.Bacc()
    q_d = nc.declare_dram_parameter("q", [128, 32768], F32, isOutput=False)
    v_d = nc.declare_dram_parameter("v", [128, 4096], F32, isOutput=False)
    A2_d = nc.declare_dram_parameter("A2", [128, 128], BF16, isOutput=False)
    cb_d = nc.declare_dram_parameter("cbias", [128, 1], F32, isOutput=False)
    vwf_d = nc.declare_dram_parameter("vwf", [128, 128], BF16, isOutput=False)
    vbp_d = nc.declare_dram_parameter("vbp", [128, 1], F32, isOutput=False)
    cwt_d = nc.declare_dram_parameter("cwt8", [128, 1536], mybir.dt.float8e4, isOutput=False)
    cbb_d = nc.declare_dram_parameter("cbb", [1, 128], BF16, isOutput=False)
    i64_d = nc.declare_dram_parameter("i64_2", [128, 64], BF16, isOutput=False)
    i128_d = nc.declare_dram_parameter("i128", [128, 128], BF16, isOutput=False)
    out_d = nc.declare_dram_parameter("out", [128, 32768], F32, isOutput=True)

    with tile.TileContext(nc) as tc, \
         tc.tile_pool(name="const", bufs=1) as cpool, \
         tc.tile_pool(name="vwork", bufs=1) as vpool, \
         tc.tile_pool(name="qin", bufs=9) as qin_pool, \
         tc.tile_pool(name="qsq", bufs=6) as qsq_pool, \
         tc.tile_pool(name="xh", bufs=8) as xh_pool, \
         tc.tile_pool(name="xhT", bufs=3) as xhT_pool, \
         tc.tile_pool(name="sig", bufs=4) as sig_pool, \
         tc.tile_pool(name="ring", bufs=1) as rg_pool, \
         tc.tile_pool(name="stat", bufs=8) as st_pool, \
         tc.tile_pool(name="vstat", bufs=2) as vst_pool, \
         tc.tile_pool(name="outp", bufs=3) as out_pool, \
         tc.tile_pool(name="ps_t1", bufs=2, space="PSUM") as ps_t1, \
         tc.tile_pool(name="ps_xt", bufs=2, space="PSUM") as ps_xt, \
         tc.tile_pool(name="ps_lg", bufs=2, space="PSUM") as ps_lg, \
         tc.tile_pool(name="ps_cv", bufs=2, space="PSUM") as ps_cv:

        def const_tile(shape, dtype, tag, src):
            t = cpool.tile(shape, dtype, tag=tag)
            nc.sync.dma_start(out=t, in_=src[:, :])
            return t

        A2_sb = const_tile([128, 128], BF16, "A2", A2_d)
        cb_sb = const_tile([128, 1], F32, "cb", cb_d)
        vwf_sb = const_tile([128, 128], BF16, "vwf", vwf_d)
        vbp_sb = const_tile([128, 1], F32, "vbp", vbp_d)
        cwt_sb = const_tile([128, 1536], mybir.dt.float8e4, "cwt", cwt_d)
        cbb_sb = const_tile([1, 128], BF16, "cbb", cbb_d)
        i64_sb = const_tile([128, 64], BF16, "i64", i64_d)
        i128_sb = const_tile([128, 128], BF16, "i128", i128_d)

        w128b = cpool.tile([128, 1], BF16, tag="w128b")  # 1/64 for q sumsq
        nc.vector.memset(w128b, 1.0 / 64)
        v128b = cpool.tile([128, 1], BF16, tag="v128b")  # 1/128 for v stats
        nc.vector.memset(v128b, 1.0 / 128)
        ones1x128b = cpool.tile([1, 128], BF16, tag="o1x")
        nc.vector.memset(ones1x128b, 1.0)
        ones512b = cpool.tile([1, 512], BF16, tag="o512")
        nc.vector.memset(ones512b, 1.0)
        # fp8 srow ring: 12 slots (row r -> slot r%12) + slot 12 duplicating
        # rows r%12==0 so tap pairs (11,12) stay contiguous for DoubleRow.
        ring = rg_pool.tile([128, 13 * 256], mybir.dt.float8e4, tag="ring")

        def rsqrt_pool(rr, vp, y0, t, t2):
            """rr = 1/sqrt(vp): bit trick + 1 Newton step. Scalar ALU ops on
            DVE (walrus rejects TensorScalarPtr on Pool), muls on Pool."""
            nc.vector.tensor_scalar(y0.bitcast(U32), vp.bitcast(U32), 1, None,
                                    ALU.logical_shift_right)
            nc.vector.tensor_scalar(y0.bitcast(I32), y0.bitcast(I32),
                                    -1, None, ALU.bitwise_xor)
            nc.vector.tensor_scalar(y0.bitcast(I32), y0.bitcast(I32),
                                    MAGIC + 1, None, ALU.add)
            nc.gpsimd.tensor_mul(t, y0, y0)
            nc.gpsimd.tensor_mul(t, t, vp)
            nc.vector.tensor_scalar(t2, t, -0.5, 1.5, ALU.mult, ALU.add)
            nc.gpsimd.tensor_mul(rr, y0, t2)

        # ---------------- V path (once per core) ----------------
        vraw = vpool.tile([128, 4096], F32, tag="vraw")
        vrb = vpool.tile([128, 4096], BF16, tag="vrb")
        vsq = vpool.tile([128, 4096], BF16, tag="vsq")
        vhat = vpool.tile([128, 4096], BF16, tag="vhat")
        V_sb = vpool.tile([128, 4096], F32, tag="V")
        for ch in range(8):
            sl = slice(ch * 512, (ch + 1) * 512)
            nc.sync.dma_start(out=vraw[:, sl], in_=v_d[:, sl])
            nc.gpsimd.tensor_copy(vrb[:, sl], vraw[:, sl])
            nc.scalar.activation(vsq[:, sl], vraw[:, sl], AF.Square)
            mu_ps = ps_lg.tile([1, 512], F32, tag="lg")
            nc.tensor.matmul(mu_ps, v128b, vrb[:, sl], start=True, stop=True)
            sq_ps = ps_xt.tile([1, 512], F32, tag="xt")
            nc.tensor.matmul(sq_ps, v128b, vsq[:, sl], start=True, stop=True)
            muc = vst_pool.tile([1, 512], F32, tag="vmu")
            nc.vector.tensor_copy(muc, mu_ps)
            m2 = vst_pool.tile([1, 512], F32, tag="vm2")
            nc.gpsimd.tensor_mul(m2, muc, muc)
            vpc = vst_pool.tile([1, 512], F32, tag="vvp")
            nc.vector.scalar_tensor_tensor(vpc, sq_ps, EPS, m2,
                                           ALU.add, ALU.subtract)
            ry = vst_pool.tile([1, 512], F32, tag="vry")
            rt_ = vst_pool.tile([1, 512], F32, tag="vrt")
            rt2 = vst_pool.tile([1, 512], F32, tag="vrt2")
            rrc = vst_pool.tile([1, 512], F32, tag="vrr")
            rsqrt_pool(rrc, vpc, ry, rt_, rt2)
            rrb = vst_pool.tile([1, 512], BF16, tag="vrrb")
            nc.gpsimd.tensor_copy(rrb, rrc)
            vcrb = vst_pool.tile([1, 512], BF16, tag="vcrb")
            nc.gpsimd.tensor_mul(vcrb, muc, rrc)
            rb = ps_lg.tile([128, 512], F32, tag="lg")
            nc.tensor.matmul(rb, ones1x128b, rrb, start=True, stop=True)
            cbb_ps = ps_xt.tile([128, 512], F32, tag="xt")
            nc.tensor.matmul(cbb_ps, ones1x128b, vcrb, start=True, stop=True)
            tmp = vst_pool.tile([128, 512], F32, tag="vtmp")
            nc.vector.tensor_mul(tmp, vraw[:, sl], rb)
            nc.vector.tensor_sub(vhat[:, sl], tmp, cbb_ps)
            vp_l = ps_lg.tile([128, 512], F32, tag="lg")
            nc.tensor.matmul(vp_l, vwf_sb, vhat[:, sl], start=True, stop=True)
            nc.scalar.add(V_sb[:, sl], vp_l, vbp_sb[:, 0:1])

        # ---------------- main loop ----------------
        qins = {}
        qfront = {}

        def load_2pairs(k):
            # pairs 2k, 2k+1: one [128,512] f32 load (parity-packed rows),
            # one [64,512] f32 load of the odd-row half for base-0 chunks.
            qin2 = qin_pool.tile([128, 512], F32, tag="qin")
            qinB = qin_pool.tile([64, 512], F32, tag="qinB")
            with tc.high_priority(offset=80):
                nc.sync.dma_start(out=qin2, in_=q_d[:, k * 512:(k + 1) * 512])
                nc.sync.dma_start(out=qinB,
                                  in_=q_d[64:128, k * 512:(k + 1) * 512])
            qb2 = qsq_pool.tile([128, 512], BF16, tag="qb")
            nc.gpsimd.tensor_copy(qb2, qin2)
            qbB = qsq_pool.tile([64, 512], BF16, tag="qbB")
            nc.gpsimd.tensor_copy(qbB, qinB)
            qsqA = qsq_pool.tile([64, 512], BF16, tag="qsqA")
            nc.gpsimd.tensor_mul(qsqA, qb2[0:64, :], qb2[0:64, :])
            qsqB = qsq_pool.tile([64, 512], BF16, tag="qsqB")
            nc.gpsimd.tensor_mul(qsqB, qbB, qbB)
            qfront[k] = (qin2, qb2, qbB, qsqA, qsqB)

        def attn_pair(m):
            if m % 2 == 0:
                load_2pairs(m // 2)
            qin2, qb2, qbB, qsqA, qsqB = qfront[m // 2]
            cb = (m % 2) * 256
            qins[m] = qin2[:, cb:cb + 256]
            t1 = ps_t1.tile([128, 264], F32, tag="t1")
            st4 = t1[:, 256:264]
            for c in range(4):
                csl = slice(cb + (c % 2) * 128, cb + (c % 2) * 128 + 128)
                srcq = qb2[0:64, csl] if c < 2 else qbB[:, csl]
                srcs = qsqA[:, csl] if c < 2 else qsqB[:, csl]
                nc.tensor.matmul(t1[:, c * 64:(c + 1) * 64], srcq,
                                 i64_sb[0:64, :], start=True, stop=True)
                nc.tensor.matmul(st4[:, c:c + 1], srcq, w128b[0:64, :],
                                 start=True, stop=True)
                nc.tensor.matmul(st4[:, 4 + c:5 + c], srcs, w128b[0:64, :],
                                 start=True, stop=True)
            st8 = st_pool.tile([128, 8], F32, tag="st8")
            nc.vector.tensor_copy(st8, st4)
            mu = st8[:, 0:4]
            mu2 = st_pool.tile([128, 4], F32, tag="mu2")
            nc.vector.tensor_mul(mu2, mu, mu)
            vp = st_pool.tile([128, 4], F32, tag="vp")
            nc.vector.tensor_sub(vp, st8[:, 4:8], mu2)
            y0 = st_pool.tile([128, 4], F32, tag="y0")
            t_ = st_pool.tile([128, 4], F32, tag="t_")
            t2 = st_pool.tile([128, 4], F32, tag="t2")
            rr = st_pool.tile([128, 4], F32, tag="rr")
            nc.vector.tensor_scalar(y0.bitcast(U32), vp.bitcast(U32), 1, None,
                                    ALU.logical_shift_right)
            nc.vector.tensor_scalar(y0.bitcast(I32), y0.bitcast(I32),
                                    -1, None, ALU.bitwise_xor)
            nc.vector.tensor_scalar(y0.bitcast(I32), y0.bitcast(I32),
                                    MAGIC + 1, None, ALU.add)
            nc.vector.tensor_mul(t_, y0, y0)
            nc.vector.tensor_mul(t_, t_, vp)
            nc.vector.tensor_scalar(t2, t_, -0.5, 1.5, ALU.mult, ALU.add)
            nc.vector.tensor_mul(rr, y0, t2)
            xhT_ps = ps_xt.tile([64, 512], F32, tag="xt")
            for c in range(4):
                xh = xh_pool.tile([128, 64], BF16, tag="xh")
                nc.vector.tensor_scalar(xh, t1[:, c * 64:(c + 1) * 64],
                                        mu[:, c:c + 1],
                                        rr[:, c:c + 1],
                                        ALU.subtract, ALU.mult)
                nc.tensor.matmul(xhT_ps[:, c * 128:(c + 1) * 128], xh,
                                 i128_sb, start=True, stop=True)
            xhT = xhT_pool.tile([64, 512], BF16, tag="xhT")
            nc.scalar.copy(xhT, xhT_ps)
            lg = ps_lg.tile([128, 512], F32, tag="lg")
            nc.tensor.matmul(lg, A2_sb[0:64, :], xhT, start=True, stop=True)
            sig = sig_pool.tile([128, 512], BF16, tag="sig")
            nc.scalar.activation(sig[:, 0:256], lg[:, 0:256], AF.Sigmoid,
                                 bias=cb_sb[:, 0:1])
            nc.scalar.activation(sig[:, 256:512], lg[:, 256:512], AF.Sigmoid,
                                 bias=cb_sb[:, 0:1])
            hy = m // 2
            vsl = V_sb[:, hy * 64:(hy + 1) * 64]
            vb_ap = vsl.rearrange("p c -> p c ()").broadcast_to([128, 64, 4])
            for r in range(2):
                row = 2 * m + r
                sig_ap = sig[:, r * 256:(r + 1) * 256].rearrange(
                    "p (c f) -> p c f", f=4)
                slots = [row % 12] + ([12] if row % 12 == 0 else [])
                for s in slots:
                    nc.gpsimd.tensor_mul(
                        ring[:, s * 256:(s + 1) * 256].rearrange(
                            "p (c f) -> p c f", f=4),
                        sig_ap, vb_ap)

        def conv_block(y0):
            cv = ps_cv.tile([128, 512], F32, tag="cv")
            nc.tensor.matmul(cv, cbb_sb, ones512b, start=True, stop=False)
            for bi, dx in enumerate((1, 0, 2)):
                for tp in range(2):
                    pb = (bi * 2 + tp) * 256
                    last = (dx == 2 and tp == 1)
                    for p in range(2):
                        rA = y0 + 2 * p - 1 + 2 * tp
                        base = p * 256
                        if rA < 0 or rA + 1 > 255:
                            # image edge: single valid tap as a plain fp8 mm
                            k = 1 if rA < 0 else 0
                            row = rA + k
                            wt = cwt_sb[:, pb + k * 128:pb + (k + 1) * 128]
                            s = row % 12
                            rt = ring[:, s * 256:(s + 1) * 256]
                            if dx == 1:
                                nc.tensor.matmul(cv[:, base:base + 256], wt,
                                                 rt[:, 0:256], start=False,
                                                 stop=last)
                            elif dx == 0:
                                nc.tensor.matmul(cv[:, base + 1:base + 256],
                                                 wt, rt[:, 0:255], start=False,
                                                 stop=False)
                            else:
                                nc.tensor.matmul(cv[:, base:base + 255], wt,
                                                 rt[:, 1:256], start=False,
                                                 stop=last)
                            continue
                        sA = rA % 12
                        lhsT = cwt_sb[:, pb:pb + 256].rearrange(
                            "p (k m) -> p k m", k=2)
                        rhs2 = ring[:, sA * 256:sA * 256 + 512].rearrange(
                            "p (k n) -> p k n", k=2)
                        DR = mybir.MatmulPerfMode.DoubleRow
                        if dx == 1:
                            nc.tensor.matmul(cv[:, base:base + 256], lhsT,
                                             rhs2, start=False, stop=False,
                                             perf_mode=DR)
                        elif dx == 0:
                            nc.tensor.matmul(cv[:, base + 1:base + 256], lhsT,
                                             rhs2[:, :, 0:255], start=False,
                                             stop=False, perf_mode=DR)
                        else:
                            nc.tensor.matmul(cv[:, base:base + 255], lhsT,
                                             rhs2[:, :, 1:256], start=False,
                                             stop=last, perf_mode=DR)
            k = y0 // 4
            ot = out_pool.tile([128, 512], F32, tag="ot")
            for p in range(2):
                m = y0 // 2 + p
                qin_m = qins.pop(m)
                nc.vector.tensor_add(ot[:, p * 256:(p + 1) * 256],
                                     cv[:, p * 256:(p + 1) * 256], qin_m)
            nc.sync.dma_start(out=out_d[:, k * 512:(k + 1) * 512], in_=ot)
            del qfront[k]

        for pi in range(131):
            if pi < 128:
                attn_pair(pi)
            if pi >= 3 and pi % 2 == 1:
                conv_block(2 * pi - 6)

    nc.finalize()
    return nc


def _fold_weights(qW, qb, vW, vb, K, qn_g, qn_b, vn_g, vn_b, cW, cb):
    f = np.float32
    qW, qb, vW, vb, K = f(qW), f(qb), f(vW), f(vb), f(K)
    qn_g, qn_b, vn_g, vn_b, cW, cb = f(qn_g), f(qn_b), f(vn_g), f(vn_b), f(cW), f(cb)
    scale = np.float32(64.0 ** -0.5)
    qWf = qn_g[:, None] * qW.T                      # [c, co]
    bprime = qb + qW @ qn_b                         # [64]
    A = scale * (qWf @ K.T)                         # [64, 128]
    c_b = scale * (K @ bprime)                      # [128]
    vWf = vn_g[:, None] * vW.T / 32.0               # [128, 128] (1/32 for fp8)
    vbp = (vb + vW @ vn_b) / 32.0                   # [128]
    cwt = np.zeros((128, 12, 128), np.float32)
    for bi, dx in enumerate((1, 0, 2)):
        for ti, t in enumerate((-1, 0, 1, 2)):
            blk = bi * 4 + ti
            if 0 <= t + 1 <= 2:
                cwt[:, blk, 0:64] = cW[:, :, t + 1, dx].T
            if 0 <= t <= 2:
                cwt[:, blk, 64:128] = cW[:, :, t, dx].T
    # fp8 DoubleRow layout: [128, 3dx, 2 tap-pairs, 2 k-tiles, 128], x32 to
    # sit in fp8e4m3's normal range (V is scaled by 1/32 to compensate).
    cwt8 = (cwt.reshape(128, 3, 2, 2, 128) * 32.0).astype(
        ml_dtypes.float8_e4m3)
    i64_2 = np.zeros((128, 64), np.float32)
    i64_2[0:64] = np.eye(64, dtype=np.float32)
    i64_2[64:128] = np.eye(64, dtype=np.float32)
    return {
        "A2": np.ascontiguousarray(
            np.concatenate([A, A], axis=0).astype(NPBF16)),
        "cbias": np.ascontiguousarray(c_b.reshape(128, 1)),
        "vwf": np.ascontiguousarray(vWf.astype(NPBF16)),
        "vbp": np.ascontiguousarray(vbp.reshape(128, 1)),
        "cwt8": np.ascontiguousarray(cwt8.reshape(128, 1536)),
        "cbb": np.ascontiguousarray(np.concatenate([cb, cb]).reshape(1, 128).astype(NPBF16)),
        "i64_2": np.ascontiguousarray(i64_2.astype(NPBF16)),
        "i128": np.ascontiguousarray(np.eye(128, dtype=np.float32).astype(NPBF16)),
    }


def _pack_q(qi):
    """[64,256,256] f32 -> [128,32768]: partitions (ch, row-parity)."""
    qs = np.empty((128, 128, 256), np.float32)
    qs[0:64] = qi[:, 0::2, :]
    qs[64:128] = qi[:, 1::2, :]
    return np.ascontiguousarray(qs.reshape(128, 32768))


def _unpack_out(r):
    """[128,32768] -> [64,256,256] undoing the row-parity packing."""
    arr = np.asarray(r, np.float32).reshape(128, 128, 256)
    out = np.empty((64, 256, 256), np.float32)
    out[:, 0::2, :] = arr[0:64]
    out[:, 1::2, :] = arr[64:128]
    return out


def _run(in_maps, trace=False, **kw):
    if "nc" not in _CACHE:
        _CACHE["nc"] = _build_nc()
    return run_bass_kernel_spmd(_CACHE["nc"], in_maps, list(range(8)),
                                trace=trace, **kw)


def kernel(q, v, qW, qb, vW, vb, K, qn_g, qn_b, vn_g, vn_b, cW, cb):
    base = _fold_weights(qW, qb, vW, vb, K, qn_g, qn_b, vn_g, vn_b, cW, cb)
    in_maps = []
    for i in range(8):
        m = dict(base)
        m["q"] = _pack_q(np.float32(q[i]))
        m["v"] = np.ascontiguousarray(np.float32(v[i]).reshape(128, 4096))
        in_maps.append(m)
    res = _run(in_maps)
    outs = [_unpack_out(r["out"]) for r in res.results]
    return np.stack(outs)



# revision 5
# speedup vs baseline: 1.8642x; 1.8642x over previous
"""LocalPatchAttention Trainium2 kernel (v2: bulk-LN restructure).

Data-parallel over batch B=8 across 8 NeuronCores (one image per core).
q and out live in DRAM as [128, 32768] with partitions = (channel,
row-parity): partition p<64 = channel p of even rows, p>=64 = channel p-64
of odd rows; host packs q to bf16 and unpacks f32 out.

Key restructure vs v1: LayerNorm statistics for ALL 256 image rows are
computed in one shot, then normalization is folded into the logits matmul
algebraically -- no per-pair transposes, no per-pair DVE stat ops.

  logits[v,px] = sum_c A[c,v]*(q[c,px]-mu[px])*rr[px]
              = A^T(q*rr) - sA[v]*(mu[px]*rr[px]),  sA = column sums of A.

Phase A (per 2-pair block k of 64): one [128,512] bf16 q DMA, Pool square,
two selector-lhsT matmuls accumulating block k's per-row mean / E[q^2]
into rows 2k:2k+2 of two shared [128,512] PSUM tiles (selector matrices
place each block's stats at its own partition pair, matmul out stays at
partition base 0).

Bulk: ~10 [128,512] DVE ops compute rr = rsqrt(var+eps) (bit-trick + 1
Newton step) and mr = mu*rr for all rows at once -> rrmr_sb bf16.

Phase B (per block): SBUF->SBUF gather DMA brings rows 2k:2k+2 of rrmr_sb
to partition base 0; PE broadcasts rr to [128,512] (parity-split ones
lhsT); one DVE multiply qs = q*rr; logits PSUM = A^T qs accumulated with
K<=2 matmuls of -sA x mr (zero-padded lhsT row selects the odd-parity mr
row); one [128,512] Sigmoid per parity (folded bias); sig*V on GPSIMD into
the fp8 ring; 3x3 conv in fp8 DoubleRow (12 matmuls per block, conv bias
folded into the residual scalar_tensor_tensor: out = (q + cb) + cv).
V path (once per core): same algebraic fold, stats batched across the 8
pixel chunks with a [128,8] selector, V = vWf^T(v*rr) - svwf x mr + vb.
"""

import numpy as np
import ml_dtypes

import concourse.bass as bass
import concourse.bacc as bacc
import concourse.tile as tile
from concourse import mybir
from concourse.bass_utils import run_bass_kernel_spmd

F32 = mybir.dt.float32
BF16 = mybir.dt.bfloat16
FP8 = mybir.dt.float8e4
U32 = mybir.dt.uint32
I32 = mybir.dt.int32
AF = mybir.ActivationFunctionType
ALU = mybir.AluOpType
EPS = 1e-5
MAGIC = 0x5F3759DF
NPBF16 = ml_dtypes.bfloat16

_CACHE = {}


def _build_nc():
    nc = bacc.Bacc()
    q_d = nc.declare_dram_parameter("q", [128, 32768], BF16, isOutput=False)
    v_d = nc.declare_dram_parameter("v", [128, 4096], BF16, isOutput=False)
    A2_d = nc.declare_dram_parameter("A2", [128, 128], BF16, isOutput=False)
    nsAe_d = nc.declare_dram_parameter("nsAe", [1, 128], BF16, isOutput=False)
    nsAo_d = nc.declare_dram_parameter("nsAo", [2, 128], BF16, isOutput=False)
    cb_d = nc.declare_dram_parameter("cbias", [128, 1], F32, isOutput=False)
    cbbp_d = nc.declare_dram_parameter("cbbp", [128, 1], F32, isOutput=False)
    vwf_d = nc.declare_dram_parameter("vwf", [128, 128], BF16, isOutput=False)
    nsv_d = nc.declare_dram_parameter("nsv", [1, 128], BF16, isOutput=False)
    vbp_d = nc.declare_dram_parameter("vbp", [128, 1], F32, isOutput=False)
    cwt_d = nc.declare_dram_parameter("cwt8", [128, 1536], FP8, isOutput=False)
    sel_d = nc.declare_dram_parameter("sel", [128, 8192], BF16, isOutput=False)
    selv_d = nc.declare_dram_parameter("selv", [128, 64], BF16, isOutput=False)
    ones2_d = nc.declare_dram_parameter("ones2", [2, 128], BF16, isOutput=False)
    out_d = nc.declare_dram_parameter("out", [128, 32768], F32, isOutput=True)

    with tile.TileContext(nc) as tc, \
         tc.tile_pool(name="const", bufs=1) as cpool, \
         tc.tile_pool(name="vwork", bufs=1) as vpool, \
         tc.tile_pool(name="qsq", bufs=4) as qsq_pool, \
         tc.tile_pool(name="bulk", bufs=1) as bk_pool, \
         tc.tile_pool(name="gath", bufs=6) as g_pool, \
         tc.tile_pool(name="qs", bufs=4) as qs_pool, \
         tc.tile_pool(name="sig", bufs=4) as sig_pool, \
         tc.tile_pool(name="ring", bufs=1) as rg_pool, \
         tc.tile_pool(name="outp", bufs=3) as out_pool:

        def const_tile(shape, dtype, tag, src):
            t = cpool.tile(shape, dtype, tag=tag)
            nc.sync.dma_start(out=t, in_=src[:, :])
            return t

        A2_sb = const_tile([128, 128], BF16, "A2", A2_d)
        nsAe_sb = const_tile([1, 128], BF16, "nsAe", nsAe_d)
        nsAo_sb = const_tile([2, 128], BF16, "nsAo", nsAo_d)
        cb_sb = const_tile([128, 1], F32, "cb", cb_d)
        cbbp_sb = const_tile([128, 1], F32, "cbbp", cbbp_d)
        vwf_sb = const_tile([128, 128], BF16, "vwf", vwf_d)
        nsv_sb = const_tile([1, 128], BF16, "nsv", nsv_d)
        vbp_sb = const_tile([128, 1], F32, "vbp", vbp_d)
        cwt_sb = const_tile([128, 1536], FP8, "cwt", cwt_d)
        sel_sb = const_tile([128, 8192], BF16, "sel", sel_d)
        selv_sb = const_tile([128, 64], BF16, "selv", selv_d)
        ones2_sb = const_tile([2, 128], BF16, "ones2", ones2_d)
        ones1 = cpool.tile([1, 128], BF16, tag="o1")
        nc.vector.memset(ones1, 1.0)

        # all of q stays resident in SBUF as bf16 (64 KB of column space)
        qall = cpool.tile([128, 32768], BF16, tag="qall")
        # bulk LN stats: cols 0:512 = rr, 512:1024 = mu*rr; partition 2k+par
        rrmr = cpool.tile([128, 1024], BF16, tag="rrmr")
        rrv = cpool.tile([8, 1024], BF16, tag="rrv")
        V_sb = cpool.tile([128, 4096], F32, tag="V")
        # fp8 srow ring: 12 slots (row r -> slot r%12) + slot 12 duplicating
        # rows r%12==0 so tap pairs (11,12) stay contiguous for DoubleRow.
        ring = rg_pool.tile([128, 13 * 256], FP8, tag="ring")

        def rsqrt_bulk(rr_out, mu_ps, sq_ps, pool, np_, tag):
            """rr_out(bf16) = 1/sqrt(sq - mu^2 + eps) on [np_,512] tiles."""
            mu2 = pool.tile([np_, 512], F32, tag=tag + "mu2")
            nc.scalar.activation(mu2, mu_ps, AF.Square)
            vp = pool.tile([np_, 512], F32, tag=tag + "vp")
            nc.vector.scalar_tensor_tensor(vp, sq_ps, EPS, mu2,
                                           ALU.add, ALU.subtract)
            y0 = pool.tile([np_, 512], F32, tag=tag + "y0")
            nc.vector.tensor_scalar(y0.bitcast(U32), vp.bitcast(U32), 1, None,
                                    ALU.logical_shift_right)
            nc.vector.tensor_scalar(y0.bitcast(I32), y0.bitcast(I32),
                                    -1, None, ALU.bitwise_xor)
            nc.vector.tensor_scalar(y0.bitcast(I32), y0.bitcast(I32),
                                    MAGIC + 1, None, ALU.add)
            t_ = pool.tile([np_, 512], F32, tag=tag + "t_")
            nc.vector.tensor_mul(t_, y0, y0)
            nc.vector.tensor_mul(t_, t_, vp)
            t2 = pool.tile([np_, 512], F32, tag=tag + "t2")
            nc.vector.tensor_scalar(t2, t_, -0.5, 1.5, ALU.mult, ALU.add)
            nc.vector.tensor_mul(rr_out, y0, t2)

        # ================= phase A: q loads + LN stats =================
        with tc.tile_pool(name="ps_mu", bufs=1, space="PSUM") as ps_mu, \
             tc.tile_pool(name="ps_sq", bufs=1, space="PSUM") as ps_sq, \
             tc.tile_pool(name="ps_v", bufs=2, space="PSUM") as ps_v:

            mu_ps = ps_mu.tile([128, 512], F32, tag="mu")
            sq_ps = ps_sq.tile([128, 512], F32, tag="sq")
            for k in range(64):
                qk = qall[:, k * 512:(k + 1) * 512]
                with tc.high_priority(offset=80):
                    nc.sync.dma_start(out=qk, in_=q_d[:, k * 512:(k + 1) * 512])
                qsq = qsq_pool.tile([128, 512], BF16, tag="qsq")
                nc.gpsimd.tensor_mul(qsq, qk, qk)
                selk = sel_sb[:, k * 128:(k + 1) * 128]
                nc.tensor.matmul(mu_ps, selk, qk,
                                 start=(k == 0), stop=(k == 63))
                nc.tensor.matmul(sq_ps, selk, qsq,
                                 start=(k == 0), stop=(k == 63))

            # ---------- V path stats (8 chunks of 512 px) ----------
            vraw = vpool.tile([128, 4096], BF16, tag="vraw")
            vsq = vpool.tile([128, 4096], BF16, tag="vsq")
            muv_ps = ps_v.tile([8, 512], F32, tag="vps")
            sqv_ps = ps_v.tile([8, 512], F32, tag="vps")
            for c in range(8):
                sl = slice(c * 512, (c + 1) * 512)
                nc.sync.dma_start(out=vraw[:, sl], in_=v_d[:, sl])
                nc.gpsimd.tensor_mul(vsq[:, sl], vraw[:, sl], vraw[:, sl])
                selc = selv_sb[:, c * 8:(c + 1) * 8]
                nc.tensor.matmul(muv_ps, selc, vraw[:, sl],
                                 start=(c == 0), stop=(c == 7))
                nc.tensor.matmul(sqv_ps, selc, vsq[:, sl],
                                 start=(c == 0), stop=(c == 7))

            # ---------- bulk rsqrt for q and v ----------
            rsqrt_bulk(rrmr[:, 0:512], mu_ps, sq_ps, bk_pool, 128, "q")
            nc.vector.tensor_mul(rrmr[:, 512:1024], mu_ps, rrmr[:, 0:512])
            rsqrt_bulk(rrv[:, 0:512], muv_ps, sqv_ps, bk_pool, 8, "v")
            nc.vector.tensor_mul(rrv[:, 512:1024], muv_ps, rrv[:, 0:512])

        # ================= phase B =================
        with tc.tile_pool(name="ps_rr", bufs=2, space="PSUM") as ps_rr, \
             tc.tile_pool(name="ps_lg", bufs=4, space="PSUM") as ps_lg, \
             tc.tile_pool(name="ps_cv", bufs=2, space="PSUM") as ps_cv:

            # ---------- V path finish ----------
            for c in range(8):
                sl = slice(c * 512, (c + 1) * 512)
                gv = g_pool.tile([1, 1024], BF16, tag="gv")
                nc.sync.dma_start(out=gv, in_=rrv[c:c + 1, :])
                rrb = ps_rr.tile([128, 512], F32, tag="rr")
                nc.tensor.matmul(rrb, ones1, gv[0:1, 0:512],
                                 start=True, stop=True)
                vs = qs_pool.tile([128, 512], BF16, tag="qs")
                nc.vector.tensor_mul(vs, vraw[:, sl], rrb)
                vl = ps_lg.tile([128, 512], F32, tag="lg")
                nc.tensor.matmul(vl, vwf_sb, vs, start=True, stop=False)
                nc.tensor.matmul(vl, nsv_sb, gv[0:1, 512:1024],
                                 start=False, stop=True)
                nc.scalar.add(V_sb[:, sl], vl, vbp_sb[:, 0:1])

            # ---------- main loop ----------
            def attn_block(k):
                gblk = g_pool.tile([2, 1024], BF16, tag="g")
                nc.sync.dma_start(out=gblk, in_=rrmr[2 * k:2 * k + 2, :])
                rr2 = ps_rr.tile([128, 512], F32, tag="rr")
                nc.tensor.matmul(rr2, ones2_sb, gblk[0:2, 0:512],
                                 start=True, stop=True)
                qs = qs_pool.tile([128, 512], BF16, tag="qs")
                nc.vector.tensor_mul(qs, qall[:, k * 512:(k + 1) * 512], rr2)
                lg_e = ps_lg.tile([128, 512], F32, tag="lg")
                nc.tensor.matmul(lg_e, A2_sb[0:64, :], qs[0:64, :],
                                 start=True, stop=False)
                nc.tensor.matmul(lg_e, nsAe_sb, gblk[0:1, 512:1024],
                                 start=False, stop=True)
                lg_o = ps_lg.tile([128, 512], F32, tag="lg")
                nc.tensor.matmul(lg_o, A2_sb[64:128, :], qs[64:128, :],
                                 start=True, stop=False)
                nc.tensor.matmul(lg_o, nsAo_sb, gblk[0:2, 512:1024],
                                 start=False, stop=True)
                sig_e = sig_pool.tile([128, 512], BF16, tag="sig")
                nc.scalar.activation(sig_e, lg_e, AF.Sigmoid,
                                     bias=cb_sb[:, 0:1])
                sig_o = sig_pool.tile([128, 512], BF16, tag="sig")
                nc.scalar.activation(sig_o, lg_o, AF.Sigmoid,
                                     bias=cb_sb[:, 0:1])
                vsl = V_sb[:, k * 64:(k + 1) * 64]
                vb_ap = vsl.rearrange("p c -> p c ()").broadcast_to([128, 64, 4])
                for r, (st, half) in zip(
                        (4 * k, 4 * k + 1, 4 * k + 2, 4 * k + 3),
                        ((sig_e, 0), (sig_o, 0), (sig_e, 1), (sig_o, 1))):
                    sig_ap = st[:, half * 256:(half + 1) * 256].rearrange(
                        "p (c f) -> p c f", f=4)
                    slots = [r % 12] + ([12] if r % 12 == 0 else [])
                    for s in slots:
                        nc.gpsimd.tensor_mul(
                            ring[:, s * 256:(s + 1) * 256].rearrange(
                                "p (c f) -> p c f", f=4),
                            sig_ap, vb_ap)

            def conv_block(y0):
                cv = ps_cv.tile([128, 512], F32, tag="cv")
                for bi, dx in enumerate((1, 0, 2)):
                    for tp in range(2):
                        pb = (bi * 2 + tp) * 256
                        last = (dx == 2 and tp == 1)
                        for p in range(2):
                            first = (bi == 0 and tp == 0 and p == 0)
                            rA = y0 + 2 * p - 1 + 2 * tp
                            base = p * 256
                            if rA < 0 or rA + 1 > 255:
                                # image edge: single valid tap, plain fp8 mm
                                kk = 1 if rA < 0 else 0
                                row = rA + kk
                                wt = cwt_sb[:, pb + kk * 128:pb + (kk + 1) * 128]
                                s = row % 12
                                rt = ring[:, s * 256:(s + 1) * 256]
                                if dx == 1:
                                    nc.tensor.matmul(cv[:, base:base + 256],
                                                     wt, rt[:, 0:256],
                                                     start=first, stop=last)
                                elif dx == 0:
                                    nc.tensor.matmul(cv[:, base + 1:base + 256],
                                                     wt, rt[:, 0:255],
                                                     start=False, stop=False)
                                else:
                                    nc.tensor.matmul(cv[:, base:base + 255],
                                                     wt, rt[:, 1:256],
                                                     start=False, stop=last)
                                continue
                            sA = rA % 12
                            lhsT = cwt_sb[:, pb:pb + 256].rearrange(
                                "p (k m) -> p k m", k=2)
                            rhs2 = ring[:, sA * 256:sA * 256 + 512].rearrange(
                                "p (k n) -> p k n", k=2)
                            DR = mybir.MatmulPerfMode.DoubleRow
                            if dx == 1:
                                nc.tensor.matmul(cv[:, base:base + 256], lhsT,
                                                 rhs2, start=first, stop=False,
                                                 perf_mode=DR)
                            elif dx == 0:
                                nc.tensor.matmul(cv[:, base + 1:base + 256],
                                                 lhsT, rhs2[:, :, 0:255],
                                                 start=False, stop=False,
                                                 perf_mode=DR)
                            else:
                                nc.tensor.matmul(cv[:, base:base + 255], lhsT,
                                                 rhs2[:, :, 1:256], start=False,
                                                 stop=last, perf_mode=DR)
                k = y0 // 4
                ot = out_pool.tile([128, 512], F32, tag="ot")
                # out = (q + conv_bias) + cv   (conv bias folded here)
                nc.vector.scalar_tensor_tensor(
                    ot, qall[:, k * 512:(k + 1) * 512], cbbp_sb[:, 0:1], cv,
                    ALU.add, ALU.add)
                nc.sync.dma_start(out=out_d[:, k * 512:(k + 1) * 512], in_=ot)

            for k in range(64):
                attn_block(k)
                if k >= 1:
                    conv_block(4 * (k - 1))
            conv_block(4 * 63)

    nc.finalize()
    return nc


def _fold_weights(qW, qb, vW, vb, K, qn_g, qn_b, vn_g, vn_b, cW, cb):
    f = np.float32
    qW, qb, vW, vb, K = f(qW), f(qb), f(vW), f(vb), f(K)
    qn_g, qn_b, vn_g, vn_b, cW, cb = f(qn_g), f(qn_b), f(vn_g), f(vn_b), f(cW), f(cb)
    scale = np.float32(64.0 ** -0.5)
    qWf = qn_g[:, None] * qW.T                      # [c, co]
    bprime = qb + qW @ qn_b                         # [64]
    A = scale * (qWf @ K.T)                         # [64, 128]
    c_b = scale * (K @ bprime)                      # [128]  (sigmoid bias)
    sA = A.sum(axis=0)                              # [128]
    nsAo = np.zeros((2, 128), np.float32)
    nsAo[1] = -sA
    vWf = vn_g[:, None] * vW.T / 32.0               # [128, 128] (1/32 for fp8)
    vbp = (vb + vW @ vn_b) / 32.0                   # [128]
    nsv = -vWf.sum(axis=0).reshape(1, 128)          # [1, 128]
    cwt = np.zeros((128, 12, 128), np.float32)
    for bi, dx in enumerate((1, 0, 2)):
        for ti, t in enumerate((-1, 0, 1, 2)):
            blk = bi * 4 + ti
            if 0 <= t + 1 <= 2:
                cwt[:, blk, 0:64] = cW[:, :, t + 1, dx].T
            if 0 <= t <= 2:
                cwt[:, blk, 64:128] = cW[:, :, t, dx].T
    # fp8 DoubleRow layout: [128, 3dx, 2 tap-pairs, 2 k-tiles, 128], x32 to
    # sit in fp8e4m3's normal range (V is scaled by 1/32 to compensate).
    cwt8 = (cwt.reshape(128, 3, 2, 2, 128) * 32.0).astype(
        ml_dtypes.float8_e4m3)
    # selector matrices: block k -> stats rows 2k (even), 2k+1 (odd)
    sel = np.zeros((128, 64, 128), np.float32)
    for k in range(64):
        sel[0:64, k, 2 * k] = 1.0 / 64
        sel[64:128, k, 2 * k + 1] = 1.0 / 64
    selv = np.zeros((128, 8, 8), np.float32)
    for c in range(8):
        selv[:, c, c] = 1.0 / 128
    ones2 = np.zeros((2, 128), np.float32)
    ones2[0, 0:64] = 1.0
    ones2[1, 64:128] = 1.0
    return {
        "A2": np.ascontiguousarray(
            np.concatenate([A, A], axis=0).astype(NPBF16)),
        "nsAe": np.ascontiguousarray((-sA).reshape(1, 128).astype(NPBF16)),
        "nsAo": np.ascontiguousarray(nsAo.astype(NPBF16)),
        "cbias": np.ascontiguousarray(c_b.reshape(128, 1)),
        "cbbp": np.ascontiguousarray(
            np.concatenate([cb, cb]).reshape(128, 1)),
        "vwf": np.ascontiguousarray(vWf.astype(NPBF16)),
        "nsv": np.ascontiguousarray(nsv.astype(NPBF16)),
        "vbp": np.ascontiguousarray(vbp.reshape(128, 1)),
        "cwt8": np.ascontiguousarray(cwt8.reshape(128, 1536)),
        "sel": np.ascontiguousarray(
            sel.reshape(128, 8192).astype(NPBF16)),
        "selv": np.ascontiguousarray(
            selv.reshape(128, 64).astype(NPBF16)),
        "ones2": np.ascontiguousarray(ones2.astype(NPBF16)),
    }


def _pack_q(qi):
    """[64,256,256] f32 -> [128,32768] bf16: partitions (ch, row-parity)."""
    qs = np.empty((128, 128, 256), np.float32)
    qs[0:64] = qi[:, 0::2, :]
    qs[64:128] = qi[:, 1::2, :]
    return np.ascontiguousarray(qs.reshape(128, 32768).astype(NPBF16))


def _unpack_out(r):
    """[128,32768] -> [64,256,256] undoing the row-parity packing."""
    arr = np.asarray(r, np.float32).reshape(128, 128, 256)
    out = np.empty((64, 256, 256), np.float32)
    out[:, 0::2, :] = arr[0:64]
    out[:, 1::2, :] = arr[64:128]
    return out


def _run(in_maps, trace=False, **kw):
    if "nc" not in _CACHE:
        _CACHE["nc"] = _build_nc()
    return run_bass_kernel_spmd(_CACHE["nc"], in_maps, list(range(8)),
                                trace=trace, **kw)


def kernel(q, v, qW, qb, vW, vb, K, qn_g, qn_b, vn_g, vn_b, cW, cb):
    base = _fold_weights(qW, qb, vW, vb, K, qn_g, qn_b, vn_g, vn_b, cW, cb)
    in_maps = []
    for i in range(8):
        m = dict(base)
        m["q"] = _pack_q(np.float32(q[i]))
        m["v"] = np.ascontiguousarray(
            np.float32(v[i]).reshape(128, 4096).astype(NPBF16))
        in_maps.append(m)
    res = _run(in_maps)
    outs = [_unpack_out(r["out"]) for r in res.results]
    return np.stack(outs)
